# revision 40
# baseline (speedup 1.0000x reference)
"""Trainium2 Bass kernel for nn_CppnPotentialCA (CPPN potential cellular automaton).

Reference computation (shapes hardcoded):
  x       [1,96,96,96,9] f32   potential field
  kernels [64,5,5,5]     f32   cross-channel conv kernels (normalized by sum)
  m, s    [64]           f32   Gaussian growth center / width
  T       []             f32   temperature
  c0, c1  [64]           i32   source / target channel per kernel pair

  kn   = kernels / sum(kernels)                  (per kernel, if sum > 0)
  pot  = conv3d_valid(wrap_pad(x)[c0[p]], kn[p]) for each pair p   [64,96,96,96]
  g    = exp(-(pot-m)^2 / (2 s^2)) * 2 - 1
  out  = clip(x + segment_sum(g, c1)/T, 0, 10)

Sharding: 2D data-parallel, 4 cores over z (24 planes each) x 2 cores over y
(48 rows each); toroidal halos of 2 resolved by host-side padding.

Device mapping (per core):
  - conv as dense matmul with z-offset blocking: output partitions =
    (16 pairs sharing 2 source channels, 8 z-offsets) = 128; contraction =
    (2 ci, 12 z-planes, 5 dy) = 120 SBUF partitions; dx in {0..4} via free-dim
    AP offsets -> only 5 accumulating matmuls per PSUM tile (vs 15 for the
    z-pair scheme).  free dim = 4 y-rows x 96 = 384.
  - growth via ScalarE: u = Square(pot*a + b), g' = Exp(-u + ln(2/T)) (fp16),
    Exp batched over the 4 channel-groups.
  - segment-sum as matmul with 0/1 matrix E accumulated over the 4 channel
    groups into PSUM [ (8 target ch, 8 zo), 384 ].
  - out = clip(xmod + seg, 0, 10) on VectorE, xmod = x - cnt_c/T from host.
Channel 0 never participates (c0,c1 in 1..8); its output x[...,0] is emitted
on the host.  All numeric inputs are runtime data; the compiled program is
value-independent.
"""

import numpy as np

C = 9        # channels (channel 0 idle)
S = 96       # spatial side
P = 64       # kernel pairs
K = 5        # kernel side
PAD = 2
MAXP = 10.0
NCORES = 8
NZC = 4                   # cores along z
NYC = 2                   # cores along y
ZS = S // NZC             # z planes per core = 24
YS = S // NYC             # y rows per core = 48
ZSLAB = ZS + 2 * PAD      # padded input z planes per core = 28
YSLAB = YS + 2 * PAD      # padded input y rows per core = 52
XW = S + 2 * PAD          # padded x width = 100
ZO = 8                    # z-offsets per output block
NZB = ZS // ZO            # 3 z-chunks per core
NCG = 4                   # channel groups (2 source channels each)
TCH = 12                  # input planes per chunk = ZO + K - 1
KROW = 2 * TCH * K        # data contraction rows = 120
KR = KROW + 1             # + ones-row carrying the folded Gaussian bias
YB = 4                    # y-rows per matmul tile
NSP = YS // YB            # 12 spatial tiles per chunk
FREE = YB * S             # matmul free dim = 384
ROWLEN = YS * XW          # im2col row length = 4800


def _build_nc():
    from contextlib import ExitStack

    import concourse.bass as bass
    import concourse.tile as tile
    from concourse import bacc, mybir
    from concourse.tile import add_dep_helper

    f32 = mybir.dt.float32
    f16 = mybir.dt.float16
    AF = mybir.ActivationFunctionType
    ALU = mybir.AluOpType

    nc = bacc.Bacc("TRN2", target_bir_lowering=False, debug=False,
                   num_devices=NCORES)

    # im2col input: xim[zb*4+cg, (ci2,t,dy), 4800] where row (ci2,t,dy) =
    # wrap-padded plane (8*zb + t) of group channel ci2, y rows dy..dy+47.
    xim_d = nc.dram_tensor("xim", [NZB * NCG, KR, ROWLEN], f16,
                           kind="ExternalInput")
    xmod_d = nc.dram_tensor("xmod", [C - 1, ZS, YS, S], f32,
                            kind="ExternalInput")
    w_d = nc.dram_tensor("wmat", [KR, NCG * K, 128], f16,
                         kind="ExternalInput")
    e_d = nc.dram_tensor("emat", [128, NCG * 128], f16, kind="ExternalInput")
    out_d = nc.dram_tensor("out", [C - 1, ZS, YS, S], f32,
                           kind="ExternalOutput")

    XM_C = ZS * YS * S       # xmod/out channel stride
    XM_Z = YS * S            # xmod/out z stride = 4608

    with tile.TileContext(nc) as tc, ExitStack() as ctx:
        consts = ctx.enter_context(tc.tile_pool(name="consts", bufs=1))
        rpool = ctx.enter_context(tc.tile_pool(name="rtiles", bufs=11))
        gpool = ctx.enter_context(tc.tile_pool(name="growth", bufs=3))
        xpool = ctx.enter_context(tc.tile_pool(name="xin", bufs=2))
        opool = ctx.enter_context(tc.tile_pool(name="oout", bufs=2))
        pconv = ctx.enter_context(tc.tile_pool(name="pconv", bufs=5,
                                               space="PSUM"))
        pseg = ctx.enter_context(tc.tile_pool(name="pseg", bufs=3,
                                              space="PSUM"))

        # Load DMAs are window-chained (issue waits on the transfer 3 back)
        # so at most 3 transfers share HBM bandwidth and they complete in
        # priority order, instead of all time-sharing and finishing late.
        chain_hist = []

        def chain(inst):
            ins = getattr(inst, "ins", inst)
            if len(chain_hist) >= 3:
                add_dep_helper(ins, chain_hist[-3], sync=True,
                               reason="window-serialize load DMAs")
            chain_hist.append(ins)

        WCOLS = NCG * K * 128        # 2560
        WCH = WCOLS // 4             # w chunk: 5 weight sets

        # PE warm-up: dummy matmuls on a zeroed tile keep the PE busy during
        # the initial DMA wait so the HAM clock-gate opens before real work.
        dumw = consts.tile([128, FREE], f16)
        nc.vector.memset(dumw[:, :], 0.0)
        dpc = pconv.tile([128, FREE], f32, name="dummy", tag="pc")
        for _ in range(12):
            nc.tensor.matmul(dpc[:, :], dumw[:, 0:128], dumw[:, :],
                             start=True, stop=True)

        w_sb = consts.tile([KR, WCOLS], f16)
        e_sb = consts.tile([128, NCG * 128], f16)

        def w_chunk(c):
            return nc.sync.dma_start(
                w_sb[:, c * WCH:(c + 1) * WCH],
                bass.AP(tensor=w_d, offset=c * WCH,
                        ap=[[WCOLS, KR], [1, WCH]]))

        def rt_part(idx, rt, r0, r1):
            return nc.sync.dma_start(
                rt[:, r0:r1],
                bass.AP(tensor=xim_d, offset=idx * KR * ROWLEN + r0,
                        ap=[[ROWLEN, KR], [1, r1 - r0]]))

        def xz_part(zb, xz, q0, q1):
            # scalar HWDGE ring: issued up front, completes off the critical
            # rt chain (64-partition transfers run at half port rate)
            return nc.scalar.dma_start(
                xz[:, q0:q1],
                bass.AP(tensor=xmod_d, offset=zb * ZO * XM_Z + q0,
                        ap=[[XM_C, C - 1], [XM_Z, ZO], [1, q1 - q0]]))

        # --- issue all loads up front, window-chained in priority order ---
        rtiles = []
        for zb in range(NZB):
            row = []
            for cg in range(NCG):
                rt = rpool.tile([KR, ROWLEN], f16, name=f"rt{zb}_{cg}",
                                tag="rt")
                row.append(rt)
            rtiles.append(row)
        xzs = [xpool.tile([P, YS * S], f32, name=f"xz{zb}", tag="xz")
               for zb in range(NZB)]

        Q = ROWLEN // 4
        xz_part(0, xzs[0], 0, YS * S)
        chain(w_chunk(0))
        chain(w_chunk(1))
        chain(rt_part(0, rtiles[0][0], 0, Q))
        chain(rt_part(1, rtiles[0][1], 0, Q))
        chain(w_chunk(2))
        chain(w_chunk(3))
        chain(rt_part(2, rtiles[0][2], 0, Q))
        chain(rt_part(3, rtiles[0][3], 0, Q))
        chain(nc.sync.dma_start(e_sb[:, :], e_d.ap()))
        for q in range(1, 4):
            for cg in range(NCG):
                chain(rt_part(cg, rtiles[0][cg], q * Q, (q + 1) * Q))
        for zb in range(1, NZB):
            for cg in range(NCG):
                chain(rt_part(zb * NCG + cg, rtiles[zb][cg], 0, ROWLEN))
            chain(xz_part(zb, xzs[zb], 0, YS * S))

        for zb in range(NZB):
            rts = [rt[:, :].rearrange("p (y x) -> p y x", y=YS, x=XW)
                   for rt in rtiles[zb]]
            zoff = zb * ZO * XM_Z
            xz = xzs[zb]
            oz = opool.tile([P, YS * S], f32, name=f"oz{zb}", tag="oz")
            for sp in range(NSP):
                gt = gpool.tile([128, NCG * FREE], f16)
                erfs = []
                for cg in range(NCG):
                    pc = pconv.tile([128, FREE], f32, name=f"pc{cg}",
                                    tag="pc")
                    for dx in range(K):
                        si = cg * K + dx
                        nc.tensor.matmul(
                            pc[:, :],
                            w_sb[:, si * 128:(si + 1) * 128],
                            rts[cg][0:KR, sp * YB:(sp + 1) * YB, dx:dx + S],
                            start=(dx == 0), stop=(dx == K - 1),
                        )
                    # growth: Derivative_Erf(v) = (2/sqrt(pi)) exp(-v^2);
                    # the sqrt(pi)/T factor is folded into E.
                    erfs.append(nc.scalar.activation(
                        gt[:, cg * FREE:(cg + 1) * FREE], pc[:, :],
                        AF.Derivative_Erf,
                    ))
                ps = pseg.tile([128, FREE], f32, name="ps", tag="ps")
                for cg in range(NCG):
                    seg = nc.tensor.matmul(
                        ps[:, :],
                        e_sb[:, cg * 128:(cg + 1) * 128],
                        gt[:, cg * FREE:(cg + 1) * FREE],
                        start=(cg == 0), stop=(cg == NCG - 1),
                    )
                    # group the 4 seg matmuls: all wait for the last erf so
                    # the PE runs them back-to-back (one weight-mode switch)
                    if cg < NCG - 1:
                        add_dep_helper(getattr(seg, "ins", seg),
                                       getattr(erfs[-1], "ins", erfs[-1]),
                                       sync=True, reason="group seg matmuls")
                ysl = slice(sp * FREE, (sp + 1) * FREE)
                nc.vector.tensor_add(oz[:, ysl], ps[0:P, :], xz[:, ysl])
                nc.vector.tensor_scalar(
                    oz[:, ysl], oz[:, ysl], 0.0, MAXP,
                    op0=ALU.max, op1=ALU.min,
                )
                if sp % 3 == 2:
                    # quarter-granular store so the tail transfer is short
                    q0 = (sp - 2) * FREE
                    nc.scalar.dma_start(
                        bass.AP(tensor=out_d, offset=zoff + q0,
                                ap=[[XM_C, C - 1], [XM_Z, ZO],
                                    [1, 3 * FREE]]),
                        oz[:, q0:q0 + 3 * FREE],
                    )
    nc.compile()
    return nc


def _host_prep(x, kernels, m, s, T, c0, c1):
    x = np.asarray(x, np.float32)
    kernels = np.asarray(kernels, np.float32)
    m = np.asarray(m, np.float32)
    s = np.asarray(s, np.float32)
    Tf = np.float32(T)
    c0 = np.asarray(c0).astype(np.int64)
    c1 = np.asarray(c1).astype(np.int64)

    xt = np.ascontiguousarray(np.moveaxis(x[0], -1, 0))            # [9,96,96,96]
    ksum = kernels.sum(axis=(1, 2, 3), keepdims=True)
    kn = np.where(ksum > 0, kernels / ksum, kernels).astype(np.float32)

    xpad = np.pad(xt, ((0, 0), (PAD, PAD), (PAD, PAD), (PAD, PAD)),
                  mode="wrap").astype(np.float16)                  # [9,100,100,100]

    cnt = np.zeros(C, np.float32)
    for p in range(P):
        cnt[c1[p]] += 1.0
    xmod = (xt[1:] - (cnt[1:] / Tf)[:, None, None, None]).astype(np.float32)

    # channel groups: cg covers source channels {2cg+1, 2cg+2}; pairs within a
    # group ordered by p.  Output partition for pair index pp = pp*8 + zo.
    groups = [[p for p in range(P) if c0[p] in (2 * cg + 1, 2 * cg + 2)]
              for cg in range(NCG)]

    # Conv weights (Gaussian affine folded in): psum accumulates
    # v = a*pot + b with a = 1/(s*sqrt2), b = -m*a.  The ones-row (row KROW)
    # carries b split into f16(b) + residual across dx slots 0 and 1.
    W = np.zeros((KR, NCG * K, 128), np.float32)
    E = np.zeros((128, NCG * 128), np.float16)
    a = (1.0 / (s * np.sqrt(np.float32(2.0)))).astype(np.float32)
    b = (-m * a).astype(np.float32)
    for cg in range(NCG):
        for pp, p in enumerate(groups[cg]):
            ci2 = 0 if c0[p] == 2 * cg + 1 else 1
            for zo in range(ZO):
                part = pp * ZO + zo
                for t in range(TCH):
                    d = t - zo
                    if 0 <= d < K:
                        W[ci2 * TCH * K + t * K:ci2 * TCH * K + t * K + K,
                          cg * K:cg * K + K, part] = kn[p, d] * a[p]
                bh = np.float16(b[p])
                W[KROW, cg * K + 0, part] = bh
                W[KROW, cg * K + 1, part] = b[p] - np.float32(bh)
                # seg-sum target: (c1-1)*8 + zo; sqrt(pi)/T rescales
                # Derivative_Erf(v) = (2/sqrt(pi)) exp(-v^2) to (2/T) exp(-v^2)
                E[part, cg * 128 + (int(c1[p]) - 1) * ZO + zo] = np.float16(
                    np.sqrt(np.pi) / Tf)
    W = W.astype(np.float16)

    in_maps = []
    for zi in range(NZC):
        for yi in range(NYC):
            slab = xpad[:, ZS * zi:ZS * zi + ZSLAB,
                        YS * yi:YS * yi + YSLAB]                   # [9,28,52,100]
            xim = np.empty((NZB * NCG, KR, ROWLEN), np.float16)
            xim[:, KROW] = np.float16(1.0)
            for zb in range(NZB):
                for cg in range(NCG):
                    dst = xim[zb * NCG + cg, :KROW].reshape(2, TCH, K, ROWLEN)
                    for ci2 in range(2):
                        ch = 2 * cg + 1 + ci2
                        for dy in range(K):
                            dst[ci2, :, dy] = (
                                slab[ch, zb * ZO:zb * ZO + TCH,
                                     dy:dy + YS].reshape(TCH, ROWLEN))
            in_maps.append({
                "xim": xim,
                "xmod": np.ascontiguousarray(
                    xmod[:, ZS * zi:ZS * zi + ZS, YS * yi:YS * yi + YS]),
                "wmat": W,
                "emat": E,
            })
    return in_maps


_NC_CACHE = {}


def _get_nc():
    if "nc" not in _NC_CACHE:
        _NC_CACHE["nc"] = _build_nc()
    return _NC_CACHE["nc"]


def _gather(results, x):
    full = np.empty((C - 1, S, S, S), np.float32)
    k = 0
    for zi in range(NZC):
        for yi in range(NYC):
            full[:, ZS * zi:ZS * zi + ZS, YS * yi:YS * yi + YS] = \
                results[k]["out"]
            k += 1
    out = np.empty((1, S, S, S, C), np.float32)
    out[0, ..., 0] = np.clip(np.asarray(x, np.float32)[0, ..., 0], 0.0, MAXP)
    out[0, ..., 1:] = np.moveaxis(full, 0, -1)
    return out


def kernel(x, kernels, m, s, T, c0, c1):
    from concourse import bass_utils

    nc = _get_nc()
    in_maps = _host_prep(x, kernels, m, s, T, c0, c1)
    res = bass_utils.run_bass_kernel_spmd(nc, in_maps, list(range(NCORES)))
    return _gather(res.results, x)


# revision 44
# speedup vs baseline: 1.0022x; 1.0022x over previous
"""Trainium2 Bass kernel for nn_CppnPotentialCA (CPPN potential cellular automaton).

Reference computation (shapes hardcoded):
  x       [1,96,96,96,9] f32   potential field
  kernels [64,5,5,5]     f32   cross-channel conv kernels (normalized by sum)
  m, s    [64]           f32   Gaussian growth center / width
  T       []             f32   temperature
  c0, c1  [64]           i32   source / target channel per kernel pair

  kn   = kernels / sum(kernels)                  (per kernel, if sum > 0)
  pot  = conv3d_valid(wrap_pad(x)[c0[p]], kn[p]) for each pair p   [64,96,96,96]
  g    = exp(-(pot-m)^2 / (2 s^2)) * 2 - 1
  out  = clip(x + segment_sum(g, c1)/T, 0, 10)

Sharding: 2D data-parallel, 4 cores over z (24 planes each) x 2 cores over y
(48 rows each); toroidal halos of 2 resolved by host-side padding.

Device mapping (per core):
  - conv as dense matmul with z-offset blocking: output partitions =
    (16 pairs sharing 2 source channels, 8 z-offsets) = 128; contraction =
    (2 ci, 12 z-planes, 5 dy) = 120 SBUF partitions; dx in {0..4} via free-dim
    AP offsets -> only 5 accumulating matmuls per PSUM tile (vs 15 for the
    z-pair scheme).  free dim = 4 y-rows x 96 = 384.
  - growth via ScalarE: u = Square(pot*a + b), g' = Exp(-u + ln(2/T)) (fp16),
    Exp batched over the 4 channel-groups.
  - segment-sum as matmul with 0/1 matrix E accumulated over the 4 channel
    groups into PSUM [ (8 target ch, 8 zo), 384 ].
  - out = clip(xmod + seg, 0, 10) on VectorE, xmod = x - cnt_c/T from host.
Channel 0 never participates (c0,c1 in 1..8); its output x[...,0] is emitted
on the host.  All numeric inputs are runtime data; the compiled program is
value-independent.
"""

import numpy as np

C = 9        # channels (channel 0 idle)
S = 96       # spatial side
P = 64       # kernel pairs
K = 5        # kernel side
PAD = 2
MAXP = 10.0
NCORES = 8
NZC = 4                   # cores along z
NYC = 2                   # cores along y
ZS = S // NZC             # z planes per core = 24
YS = S // NYC             # y rows per core = 48
ZSLAB = ZS + 2 * PAD      # padded input z planes per core = 28
YSLAB = YS + 2 * PAD      # padded input y rows per core = 52
XW = S + 2 * PAD          # padded x width = 100
ZO = 8                    # z-offsets per output block
NZB = ZS // ZO            # 3 z-chunks per core
NCG = 4                   # channel groups (2 source channels each)
TCH = 12                  # input planes per chunk = ZO + K - 1
KROW = 2 * TCH * K        # data contraction rows = 120
KR = KROW + 1             # + ones-row carrying the folded Gaussian bias
YB = 4                    # y-rows per matmul tile
NSP = YS // YB            # 12 spatial tiles per chunk
FREE = YB * S             # matmul free dim = 384
ROWLEN = YS * XW          # im2col row length = 4800


def _build_nc():
    from contextlib import ExitStack

    import concourse.bass as bass
    import concourse.tile as tile
    from concourse import bacc, mybir
    from concourse.tile import add_dep_helper

    f32 = mybir.dt.float32
    f16 = mybir.dt.float16
    AF = mybir.ActivationFunctionType
    ALU = mybir.AluOpType

    nc = bacc.Bacc("TRN2", target_bir_lowering=False, debug=False,
                   num_devices=NCORES)

    # im2col input: xim[zb*4+cg, (ci2,t,dy), 4800] where row (ci2,t,dy) =
    # wrap-padded plane (8*zb + t) of group channel ci2, y rows dy..dy+47.
    xim_d = nc.dram_tensor("xim", [NZB * NCG, KR, ROWLEN], f16,
                           kind="ExternalInput")
    xmod_d = nc.dram_tensor("xmod", [C - 1, ZS, YS, S], f32,
                            kind="ExternalInput")
    w_d = nc.dram_tensor("wmat", [KR, NCG * K, 128], f16,
                         kind="ExternalInput")
    e_d = nc.dram_tensor("emat", [128, NCG * 128], f16, kind="ExternalInput")
    out_d = nc.dram_tensor("out", [C - 1, ZS, YS, S], f32,
                           kind="ExternalOutput")

    XM_C = ZS * YS * S       # xmod/out channel stride
    XM_Z = YS * S            # xmod/out z stride = 4608

    with tile.TileContext(nc) as tc, ExitStack() as ctx:
        consts = ctx.enter_context(tc.tile_pool(name="consts", bufs=1))
        rpool = ctx.enter_context(tc.tile_pool(name="rtiles", bufs=11))
        gpool = ctx.enter_context(tc.tile_pool(name="growth", bufs=3))
        xpool = ctx.enter_context(tc.tile_pool(name="xin", bufs=2))
        opool = ctx.enter_context(tc.tile_pool(name="oout", bufs=2))
        pconv = ctx.enter_context(tc.tile_pool(name="pconv", bufs=5,
                                               space="PSUM"))
        pseg = ctx.enter_context(tc.tile_pool(name="pseg", bufs=3,
                                              space="PSUM"))

        # Load DMAs are split across two independent DGE rings (sync HWDGE
        # and gpsimd SWDGE) and window-chained per ring, so transfers finish
        # in priority order at ~2x the single-ring rate.  The scalar ring is
        # kept wait-free (its queue also runs the erf activations).
        chain_hists = {"A": [], "C": []}

        def chain(ring, inst):
            ins = getattr(inst, "ins", inst)
            hist = chain_hists[ring]
            if len(hist) >= 2:
                add_dep_helper(ins, hist[-2], sync=True,
                               reason="window-serialize load DMAs")
            hist.append(ins)

        WCOLS = NCG * K * 128        # 2560
        WCH = WCOLS // 4             # w chunk: 5 weight sets

        # PE warm-up: dummy matmuls on a zeroed tile keep the PE busy during
        # the initial DMA wait so the HAM clock-gate opens before real work.
        dumw = consts.tile([128, FREE], f16)
        nc.vector.memset(dumw[:, :], 0.0)
        dpc = pconv.tile([128, FREE], f32, name="dummy", tag="pc")
        for _ in range(12):
            nc.tensor.matmul(dpc[:, :], dumw[:, 0:128], dumw[:, :],
                             start=True, stop=True)

        w_sb = consts.tile([KR, WCOLS], f16)
        e_sb = consts.tile([128, NCG * 128], f16)

        def w_chunk(c):
            return nc.sync.dma_start(
                w_sb[:, c * WCH:(c + 1) * WCH],
                bass.AP(tensor=w_d, offset=c * WCH,
                        ap=[[WCOLS, KR], [1, WCH]]))

        def rt_part(eng, idx, rt, r0, r1):
            return eng.dma_start(
                rt[:, r0:r1],
                bass.AP(tensor=xim_d, offset=idx * KR * ROWLEN + r0,
                        ap=[[ROWLEN, KR], [1, r1 - r0]]))

        def xz_part(eng, zb, xz, q0, q1):
            return eng.dma_start(
                xz[:, q0:q1],
                bass.AP(tensor=xmod_d, offset=zb * ZO * XM_Z + q0,
                        ap=[[XM_C, C - 1], [XM_Z, ZO], [1, q1 - q0]]))

        # --- issue all loads up front, window-chained in priority order ---
        rtiles = []
        for zb in range(NZB):
            row = []
            for cg in range(NCG):
                rt = rpool.tile([KR, ROWLEN], f16, name=f"rt{zb}_{cg}",
                                tag="rt")
                row.append(rt)
            rtiles.append(row)
        xzs = [xpool.tile([P, YS * S], f32, name=f"xz{zb}", tag="xz")
               for zb in range(NZB)]

        Q = ROWLEN // 4
        xz_part(nc.scalar, 0, xzs[0], 0, YS * S)
        chain("A", w_chunk(0))
        chain("A", w_chunk(1))
        chain("C", nc.gpsimd.dma_start(e_sb[:, :], e_d.ap()))
        for q in range(4):
            chain("A", rt_part(nc.sync, 0, rtiles[0][0], q * Q, (q + 1) * Q))
            chain("A", rt_part(nc.sync, 1, rtiles[0][1], q * Q, (q + 1) * Q))
            chain("C", rt_part(nc.gpsimd, 2, rtiles[0][2],
                               q * Q, (q + 1) * Q))
            chain("C", rt_part(nc.gpsimd, 3, rtiles[0][3],
                               q * Q, (q + 1) * Q))
            if q == 0:
                chain("A", w_chunk(2))
                chain("A", w_chunk(3))
        for zb in range(1, NZB):
            for cg in range(2):
                chain("A", rt_part(nc.sync, zb * NCG + cg, rtiles[zb][cg],
                                   0, ROWLEN))
            for cg in range(2, NCG):
                chain("C", rt_part(nc.gpsimd, zb * NCG + cg, rtiles[zb][cg],
                                   0, ROWLEN))
            chain("C", xz_part(nc.gpsimd, zb, xzs[zb], 0, YS * S))

        for zb in range(NZB):
            rts = [rt[:, :].rearrange("p (y x) -> p y x", y=YS, x=XW)
                   for rt in rtiles[zb]]
            zoff = zb * ZO * XM_Z
            xz = xzs[zb]
            oz = opool.tile([P, YS * S], f32, name=f"oz{zb}", tag="oz")
            for sp in range(NSP):
                gt = gpool.tile([128, NCG * FREE], f16)
                erfs = []
                for cg in range(NCG):
                    pc = pconv.tile([128, FREE], f32, name=f"pc{cg}",
                                    tag="pc")
                    for dx in range(K):
                        si = cg * K + dx
                        nc.tensor.matmul(
                            pc[:, :],
                            w_sb[:, si * 128:(si + 1) * 128],
                            rts[cg][0:KR, sp * YB:(sp + 1) * YB, dx:dx + S],
                            start=(dx == 0), stop=(dx == K - 1),
                        )
                    # growth: Derivative_Erf(v) = (2/sqrt(pi)) exp(-v^2);
                    # the sqrt(pi)/T factor is folded into E.
                    erfs.append(nc.scalar.activation(
                        gt[:, cg * FREE:(cg + 1) * FREE], pc[:, :],
                        AF.Derivative_Erf,
                    ))
                ps = pseg.tile([128, FREE], f32, name="ps", tag="ps")
                for cg in range(NCG):
                    seg = nc.tensor.matmul(
                        ps[:, :],
                        e_sb[:, cg * 128:(cg + 1) * 128],
                        gt[:, cg * FREE:(cg + 1) * FREE],
                        start=(cg == 0), stop=(cg == NCG - 1),
                    )
                    # group the 4 seg matmuls: all wait for the last erf so
                    # the PE runs them back-to-back (one weight-mode switch)
                    if cg < NCG - 1:
                        add_dep_helper(getattr(seg, "ins", seg),
                                       getattr(erfs[-1], "ins", erfs[-1]),
                                       sync=True, reason="group seg matmuls")
                ysl = slice(sp * FREE, (sp + 1) * FREE)
                nc.vector.tensor_add(oz[:, ysl], ps[0:P, :], xz[:, ysl])
                nc.vector.tensor_scalar(
                    oz[:, ysl], oz[:, ysl], 0.0, MAXP,
                    op0=ALU.max, op1=ALU.min,
                )
                if sp % 3 == 2:
                    # quarter-granular store so the tail transfer is short
                    q0 = (sp - 2) * FREE
                    nc.scalar.dma_start(
                        bass.AP(tensor=out_d, offset=zoff + q0,
                                ap=[[XM_C, C - 1], [XM_Z, ZO],
                                    [1, 3 * FREE]]),
                        oz[:, q0:q0 + 3 * FREE],
                    )
    nc.compile()
    return nc


def _host_prep(x, kernels, m, s, T, c0, c1):
    x = np.asarray(x, np.float32)
    kernels = np.asarray(kernels, np.float32)
    m = np.asarray(m, np.float32)
    s = np.asarray(s, np.float32)
    Tf = np.float32(T)
    c0 = np.asarray(c0).astype(np.int64)
    c1 = np.asarray(c1).astype(np.int64)

    xt = np.ascontiguousarray(np.moveaxis(x[0], -1, 0))            # [9,96,96,96]
    ksum = kernels.sum(axis=(1, 2, 3), keepdims=True)
    kn = np.where(ksum > 0, kernels / ksum, kernels).astype(np.float32)

    xpad = np.pad(xt, ((0, 0), (PAD, PAD), (PAD, PAD), (PAD, PAD)),
                  mode="wrap").astype(np.float16)                  # [9,100,100,100]

    cnt = np.zeros(C, np.float32)
    for p in range(P):
        cnt[c1[p]] += 1.0
    xmod = (xt[1:] - (cnt[1:] / Tf)[:, None, None, None]).astype(np.float32)

    # channel groups: cg covers source channels {2cg+1, 2cg+2}; pairs within a
    # group ordered by p.  Output partition for pair index pp = pp*8 + zo.
    groups = [[p for p in range(P) if c0[p] in (2 * cg + 1, 2 * cg + 2)]
              for cg in range(NCG)]

    # Conv weights (Gaussian affine folded in): psum accumulates
    # v = a*pot + b with a = 1/(s*sqrt2), b = -m*a.  The ones-row (row KROW)
    # carries b split into f16(b) + residual across dx slots 0 and 1.
    W = np.zeros((KR, NCG * K, 128), np.float32)
    E = np.zeros((128, NCG * 128), np.float16)
    a = (1.0 / (s * np.sqrt(np.float32(2.0)))).astype(np.float32)
    b = (-m * a).astype(np.float32)
    for cg in range(NCG):
        for pp, p in enumerate(groups[cg]):
            ci2 = 0 if c0[p] == 2 * cg + 1 else 1
            for zo in range(ZO):
                part = pp * ZO + zo
                for t in range(TCH):
                    d = t - zo
                    if 0 <= d < K:
                        W[ci2 * TCH * K + t * K:ci2 * TCH * K + t * K + K,
                          cg * K:cg * K + K, part] = kn[p, d] * a[p]
                bh = np.float16(b[p])
                W[KROW, cg * K + 0, part] = bh
                W[KROW, cg * K + 1, part] = b[p] - np.float32(bh)
                # seg-sum target: (c1-1)*8 + zo; sqrt(pi)/T rescales
                # Derivative_Erf(v) = (2/sqrt(pi)) exp(-v^2) to (2/T) exp(-v^2)
                E[part, cg * 128 + (int(c1[p]) - 1) * ZO + zo] = np.float16(
                    np.sqrt(np.pi) / Tf)
    W = W.astype(np.float16)

    in_maps = []
    for zi in range(NZC):
        for yi in range(NYC):
            slab = xpad[:, ZS * zi:ZS * zi + ZSLAB,
                        YS * yi:YS * yi + YSLAB]                   # [9,28,52,100]
            xim = np.empty((NZB * NCG, KR, ROWLEN), np.float16)
            xim[:, KROW] = np.float16(1.0)
            for zb in range(NZB):
                for cg in range(NCG):
                    dst = xim[zb * NCG + cg, :KROW].reshape(2, TCH, K, ROWLEN)
                    for ci2 in range(2):
                        ch = 2 * cg + 1 + ci2
                        for dy in range(K):
                            dst[ci2, :, dy] = (
                                slab[ch, zb * ZO:zb * ZO + TCH,
                                     dy:dy + YS].reshape(TCH, ROWLEN))
            in_maps.append({
                "xim": xim,
                "xmod": np.ascontiguousarray(
                    xmod[:, ZS * zi:ZS * zi + ZS, YS * yi:YS * yi + YS]),
                "wmat": W,
                "emat": E,
            })
    return in_maps


_NC_CACHE = {}


def _get_nc():
    if "nc" not in _NC_CACHE:
        _NC_CACHE["nc"] = _build_nc()
    return _NC_CACHE["nc"]


def _gather(results, x):
    full = np.empty((C - 1, S, S, S), np.float32)
    k = 0
    for zi in range(NZC):
        for yi in range(NYC):
            full[:, ZS * zi:ZS * zi + ZS, YS * yi:YS * yi + YS] = \
                results[k]["out"]
            k += 1
    out = np.empty((1, S, S, S, C), np.float32)
    out[0, ..., 0] = np.clip(np.asarray(x, np.float32)[0, ..., 0], 0.0, MAXP)
    out[0, ..., 1:] = np.moveaxis(full, 0, -1)
    return out


def kernel(x, kernels, m, s, T, c0, c1):
    from concourse import bass_utils

    nc = _get_nc()
    in_maps = _host_prep(x, kernels, m, s, T, c0, c1)
    res = bass_utils.run_bass_kernel_spmd(nc, in_maps, list(range(NCORES)))
    return _gather(res.results, x)


# revision 46
# speedup vs baseline: 1.0391x; 1.0369x over previous
"""Trainium2 Bass kernel for nn_CppnPotentialCA (CPPN potential cellular automaton).

Reference computation (shapes hardcoded):
  x       [1,96,96,96,9] f32   potential field
  kernels [64,5,5,5]     f32   cross-channel conv kernels (normalized by sum)
  m, s    [64]           f32   Gaussian growth center / width
  T       []             f32   temperature
  c0, c1  [64]           i32   source / target channel per kernel pair

  kn   = kernels / sum(kernels)                  (per kernel, if sum > 0)
  pot  = conv3d_valid(wrap_pad(x)[c0[p]], kn[p]) for each pair p   [64,96,96,96]
  g    = exp(-(pot-m)^2 / (2 s^2)) * 2 - 1
  out  = clip(x + segment_sum(g, c1)/T, 0, 10)

Sharding: 2D data-parallel, 4 cores over z (24 planes each) x 2 cores over y
(48 rows each); toroidal halos of 2 resolved by host-side padding.

Device mapping (per core):
  - conv as dense matmul with z-offset blocking: output partitions =
    (16 pairs sharing 2 source channels, 8 z-offsets) = 128; contraction =
    (2 ci, 12 z-planes, 5 dy) = 120 SBUF partitions; dx in {0..4} via free-dim
    AP offsets -> only 5 accumulating matmuls per PSUM tile (vs 15 for the
    z-pair scheme).  free dim = 4 y-rows x 96 = 384.
  - growth via ScalarE: u = Square(pot*a + b), g' = Exp(-u + ln(2/T)) (fp16),
    Exp batched over the 4 channel-groups.
  - segment-sum as matmul with 0/1 matrix E accumulated over the 4 channel
    groups into PSUM [ (8 target ch, 8 zo), 384 ].
  - out = clip(xmod + seg, 0, 10) on VectorE, xmod = x - cnt_c/T from host.
Channel 0 never participates (c0,c1 in 1..8); its output x[...,0] is emitted
on the host.  All numeric inputs are runtime data; the compiled program is
value-independent.
"""

import numpy as np

C = 9        # channels (channel 0 idle)
S = 96       # spatial side
P = 64       # kernel pairs
K = 5        # kernel side
PAD = 2
MAXP = 10.0
NCORES = 8
NZC = 4                   # cores along z
NYC = 2                   # cores along y
ZS = S // NZC             # z planes per core = 24
YS = S // NYC             # y rows per core = 48
ZSLAB = ZS + 2 * PAD      # padded input z planes per core = 28
YSLAB = YS + 2 * PAD      # padded input y rows per core = 52
XW = S + 2 * PAD          # padded x width = 100
ZO = 8                    # z-offsets per output block
NZB = ZS // ZO            # 3 z-chunks per core
NCG = 4                   # channel groups (2 source channels each)
TCH = 12                  # input planes per chunk = ZO + K - 1
KROW = 2 * TCH * K        # data contraction rows = 120
KR = KROW + 1             # + ones-row carrying the folded Gaussian bias
YB = 4                    # y-rows per matmul tile
NSP = YS // YB            # 12 spatial tiles per chunk
FREE = YB * S             # matmul free dim = 384
ROWLEN = YS * XW          # im2col row length = 4800


def _build_nc():
    from contextlib import ExitStack

    import concourse.bass as bass
    import concourse.tile as tile
    from concourse import bacc, mybir
    from concourse.tile import add_dep_helper

    f32 = mybir.dt.float32
    f16 = mybir.dt.float16
    AF = mybir.ActivationFunctionType
    ALU = mybir.AluOpType

    nc = bacc.Bacc("TRN2", target_bir_lowering=False, debug=False,
                   num_devices=NCORES)

    # im2col input: xim[zb*4+cg, (ci2,t,dy), 4800] where row (ci2,t,dy) =
    # wrap-padded plane (8*zb + t) of group channel ci2, y rows dy..dy+47.
    xim_d = nc.dram_tensor("xim", [NZB * NCG, KR, ROWLEN], f16,
                           kind="ExternalInput")
    xmod_d = nc.dram_tensor("xmod", [C - 1, ZS, YS, S], f32,
                            kind="ExternalInput")
    w_d = nc.dram_tensor("wmat", [KR, NCG * K, 128], f16,
                         kind="ExternalInput")
    e_d = nc.dram_tensor("emat", [128, NCG * 128], f16, kind="ExternalInput")
    out_d = nc.dram_tensor("out", [C - 1, ZS, YS, S], f32,
                           kind="ExternalOutput")

    XM_C = ZS * YS * S       # xmod/out channel stride
    XM_Z = YS * S            # xmod/out z stride = 4608

    with tile.TileContext(nc) as tc, ExitStack() as ctx:
        consts = ctx.enter_context(tc.tile_pool(name="consts", bufs=1))
        rpool = ctx.enter_context(tc.tile_pool(name="rtiles", bufs=11))
        gpool = ctx.enter_context(tc.tile_pool(name="growth", bufs=3))
        xpool = ctx.enter_context(tc.tile_pool(name="xin", bufs=2))
        opool = ctx.enter_context(tc.tile_pool(name="oout", bufs=2))
        pconv = ctx.enter_context(tc.tile_pool(name="pconv", bufs=5,
                                               space="PSUM"))
        pseg = ctx.enter_context(tc.tile_pool(name="pseg", bufs=3,
                                              space="PSUM"))

        # Load DMAs are split across two independent DGE rings (sync HWDGE
        # and gpsimd SWDGE) and window-chained per ring, so transfers finish
        # in priority order at ~2x the single-ring rate.  The scalar ring is
        # kept wait-free (its queue also runs the erf activations).
        chain_hists = {"A": [], "C": []}

        def chain(ring, inst):
            ins = getattr(inst, "ins", inst)
            hist = chain_hists[ring]
            if len(hist) >= 2:
                add_dep_helper(ins, hist[-2], sync=True,
                               reason="window-serialize load DMAs")
            hist.append(ins)

        WCOLS = NCG * K * 128        # 2560
        WCH = WCOLS // 4             # w chunk: 5 weight sets

        # PE warm-up: dummy matmuls on a zeroed tile keep the PE busy during
        # the initial DMA wait so the HAM clock-gate opens before real work.
        dumw = consts.tile([128, FREE], f16)
        nc.vector.memset(dumw[:, :], 0.0)
        dpc = pconv.tile([128, FREE], f32, name="dummy", tag="pc")
        for _ in range(12):
            nc.tensor.matmul(dpc[:, :], dumw[:, 0:128], dumw[:, :],
                             start=True, stop=True)

        w_sb = consts.tile([KR, WCOLS], f16)
        e_sb = consts.tile([128, NCG * 128], f16)

        def w_chunk(c):
            return nc.sync.dma_start(
                w_sb[:, c * WCH:(c + 1) * WCH],
                bass.AP(tensor=w_d, offset=c * WCH,
                        ap=[[WCOLS, KR], [1, WCH]]))

        def rt_part(eng, idx, rt, r0, r1):
            return eng.dma_start(
                rt[:, r0:r1],
                bass.AP(tensor=xim_d, offset=idx * KR * ROWLEN + r0,
                        ap=[[ROWLEN, KR], [1, r1 - r0]]))

        def xz_part(eng, zb, xz, q0, q1):
            return eng.dma_start(
                xz[:, q0:q1],
                bass.AP(tensor=xmod_d, offset=zb * ZO * XM_Z + q0,
                        ap=[[XM_C, C - 1], [XM_Z, ZO], [1, q1 - q0]]))

        # --- issue all loads up front, window-chained in priority order ---
        rtiles = []
        for zb in range(NZB):
            row = []
            for cg in range(NCG):
                rt = rpool.tile([KR, ROWLEN], f16, name=f"rt{zb}_{cg}",
                                tag="rt")
                row.append(rt)
            rtiles.append(row)
        xzs = [xpool.tile([P, YS * S], f32, name=f"xz{zb}", tag="xz")
               for zb in range(NZB)]

        Q = ROWLEN // 4
        xz_part(nc.scalar, 0, xzs[0], 0, YS * S)
        # ring A: weight chunks first (small), then even-q needs; ring C
        # starts with the very first compute dependency rtq0(cg0).
        chain("C", rt_part(nc.gpsimd, 0, rtiles[0][0], 0, Q))
        chain("A", w_chunk(0))
        chain("A", w_chunk(1))
        chain("C", rt_part(nc.gpsimd, 1, rtiles[0][1], 0, Q))
        chain("A", w_chunk(2))
        chain("A", w_chunk(3))
        chain("C", rt_part(nc.gpsimd, 2, rtiles[0][2], 0, Q))
        chain("A", rt_part(nc.sync, 3, rtiles[0][3], 0, Q))
        chain("C", nc.gpsimd.dma_start(e_sb[:, :], e_d.ap()))
        for q in range(1, 4):
            chain("C", rt_part(nc.gpsimd, 0, rtiles[0][0], q * Q,
                               (q + 1) * Q))
            chain("A", rt_part(nc.sync, 1, rtiles[0][1], q * Q, (q + 1) * Q))
            chain("C", rt_part(nc.gpsimd, 2, rtiles[0][2], q * Q,
                               (q + 1) * Q))
            chain("A", rt_part(nc.sync, 3, rtiles[0][3], q * Q, (q + 1) * Q))
        for zb in range(1, NZB):
            for cg in range(2):
                chain("A", rt_part(nc.sync, zb * NCG + cg, rtiles[zb][cg],
                                   0, ROWLEN))
            for cg in range(2, NCG):
                chain("C", rt_part(nc.gpsimd, zb * NCG + cg, rtiles[zb][cg],
                                   0, ROWLEN))
            chain("C", xz_part(nc.gpsimd, zb, xzs[zb], 0, YS * S))

        def conv_erf(rts, gt, sp, cg):
            pc = pconv.tile([128, FREE], f32, name=f"pc{cg}", tag="pc")
            for dx in range(K):
                si = cg * K + dx
                nc.tensor.matmul(
                    pc[:, :],
                    w_sb[:, si * 128:(si + 1) * 128],
                    rts[cg][0:KR, sp * YB:(sp + 1) * YB, dx:dx + S],
                    start=(dx == 0), stop=(dx == K - 1),
                )
            # growth: Derivative_Erf(v) = (2/sqrt(pi)) exp(-v^2);
            # the sqrt(pi)/T factor is folded into E.
            return nc.scalar.activation(
                gt[:, cg * FREE:(cg + 1) * FREE], pc[:, :], AF.Derivative_Erf)

        def seg_out(gt, last_erf, oz, xz, zoff, sp):
            ps = pseg.tile([128, FREE], f32, name="ps", tag="ps")
            for cg in range(NCG):
                seg = nc.tensor.matmul(
                    ps[:, :],
                    e_sb[:, cg * 128:(cg + 1) * 128],
                    gt[:, cg * FREE:(cg + 1) * FREE],
                    start=(cg == 0), stop=(cg == NCG - 1),
                )
                # group the 4 seg matmuls: all wait for the last erf so
                # the PE runs them back-to-back (one weight-mode switch)
                if cg < NCG - 1:
                    add_dep_helper(getattr(seg, "ins", seg),
                                   getattr(last_erf, "ins", last_erf),
                                   sync=True, reason="group seg matmuls")
            ysl = slice(sp * FREE, (sp + 1) * FREE)
            nc.vector.tensor_add(oz[:, ysl], ps[0:P, :], xz[:, ysl])
            nc.vector.tensor_scalar(
                oz[:, ysl], oz[:, ysl], 0.0, MAXP,
                op0=ALU.max, op1=ALU.min,
            )
            if sp % 3 == 2:
                # quarter-granular store so the tail transfer is short
                q0 = (sp - 2) * FREE
                nc.scalar.dma_start(
                    bass.AP(tensor=out_d, offset=zoff + q0,
                            ap=[[XM_C, C - 1], [XM_Z, ZO], [1, 3 * FREE]]),
                    oz[:, q0:q0 + 3 * FREE],
                )

        for zb in range(NZB):
            rts = [rt[:, :].rearrange("p (y x) -> p y x", y=YS, x=XW)
                   for rt in rtiles[zb]]
            zoff = zb * ZO * XM_Z
            xz = xzs[zb]
            oz = opool.tile([P, YS * S], f32, name=f"oz{zb}", tag="oz")
            if zb == 0:
                # cg-major within each quarter: compute starts as soon as
                # the first weight chunk + first tile quarter have landed
                for q in range(4):
                    sps = range(3 * q, 3 * q + 3)
                    gts = {sp: gpool.tile([128, NCG * FREE], f16,
                                          name=f"gt{sp}", tag="gt")
                           for sp in sps}
                    last = {}
                    for cg in range(NCG):
                        for sp in sps:
                            last[sp] = conv_erf(rts, gts[sp], sp, cg)
                    for sp in sps:
                        seg_out(gts[sp], last[sp], oz, xz, zoff, sp)
            else:
                for sp in range(NSP):
                    gt = gpool.tile([128, NCG * FREE], f16, name=f"gt{sp}",
                                    tag="gt")
                    last = None
                    for cg in range(NCG):
                        last = conv_erf(rts, gt, sp, cg)
                    seg_out(gt, last, oz, xz, zoff, sp)
    nc.compile()
    return nc


def _host_prep(x, kernels, m, s, T, c0, c1):
    x = np.asarray(x, np.float32)
    kernels = np.asarray(kernels, np.float32)
    m = np.asarray(m, np.float32)
    s = np.asarray(s, np.float32)
    Tf = np.float32(T)
    c0 = np.asarray(c0).astype(np.int64)
    c1 = np.asarray(c1).astype(np.int64)

    xt = np.ascontiguousarray(np.moveaxis(x[0], -1, 0))            # [9,96,96,96]
    ksum = kernels.sum(axis=(1, 2, 3), keepdims=True)
    kn = np.where(ksum > 0, kernels / ksum, kernels).astype(np.float32)

    xpad = np.pad(xt, ((0, 0), (PAD, PAD), (PAD, PAD), (PAD, PAD)),
                  mode="wrap").astype(np.float16)                  # [9,100,100,100]

    cnt = np.zeros(C, np.float32)
    for p in range(P):
        cnt[c1[p]] += 1.0
    xmod = (xt[1:] - (cnt[1:] / Tf)[:, None, None, None]).astype(np.float32)

    # channel groups: cg covers source channels {2cg+1, 2cg+2}; pairs within a
    # group ordered by p.  Output partition for pair index pp = pp*8 + zo.
    groups = [[p for p in range(P) if c0[p] in (2 * cg + 1, 2 * cg + 2)]
              for cg in range(NCG)]

    # Conv weights (Gaussian affine folded in): psum accumulates
    # v = a*pot + b with a = 1/(s*sqrt2), b = -m*a.  The ones-row (row KROW)
    # carries b split into f16(b) + residual across dx slots 0 and 1.
    W = np.zeros((KR, NCG * K, 128), np.float32)
    E = np.zeros((128, NCG * 128), np.float16)
    a = (1.0 / (s * np.sqrt(np.float32(2.0)))).astype(np.float32)
    b = (-m * a).astype(np.float32)
    for cg in range(NCG):
        for pp, p in enumerate(groups[cg]):
            ci2 = 0 if c0[p] == 2 * cg + 1 else 1
            for zo in range(ZO):
                part = pp * ZO + zo
                for t in range(TCH):
                    d = t - zo
                    if 0 <= d < K:
                        W[ci2 * TCH * K + t * K:ci2 * TCH * K + t * K + K,
                          cg * K:cg * K + K, part] = kn[p, d] * a[p]
                bh = np.float16(b[p])
                W[KROW, cg * K + 0, part] = bh
                W[KROW, cg * K + 1, part] = b[p] - np.float32(bh)
                # seg-sum target: (c1-1)*8 + zo; sqrt(pi)/T rescales
                # Derivative_Erf(v) = (2/sqrt(pi)) exp(-v^2) to (2/T) exp(-v^2)
                E[part, cg * 128 + (int(c1[p]) - 1) * ZO + zo] = np.float16(
                    np.sqrt(np.pi) / Tf)
    W = W.astype(np.float16)

    in_maps = []
    for zi in range(NZC):
        for yi in range(NYC):
            slab = xpad[:, ZS * zi:ZS * zi + ZSLAB,
                        YS * yi:YS * yi + YSLAB]                   # [9,28,52,100]
            xim = np.empty((NZB * NCG, KR, ROWLEN), np.float16)
            xim[:, KROW] = np.float16(1.0)
            for zb in range(NZB):
                for cg in range(NCG):
                    dst = xim[zb * NCG + cg, :KROW].reshape(2, TCH, K, ROWLEN)
                    for ci2 in range(2):
                        ch = 2 * cg + 1 + ci2
                        for dy in range(K):
                            dst[ci2, :, dy] = (
                                slab[ch, zb * ZO:zb * ZO + TCH,
                                     dy:dy + YS].reshape(TCH, ROWLEN))
            in_maps.append({
                "xim": xim,
                "xmod": np.ascontiguousarray(
                    xmod[:, ZS * zi:ZS * zi + ZS, YS * yi:YS * yi + YS]),
                "wmat": W,
                "emat": E,
            })
    return in_maps


_NC_CACHE = {}


def _get_nc():
    if "nc" not in _NC_CACHE:
        _NC_CACHE["nc"] = _build_nc()
    return _NC_CACHE["nc"]


def _gather(results, x):
    full = np.empty((C - 1, S, S, S), np.float32)
    k = 0
    for zi in range(NZC):
        for yi in range(NYC):
            full[:, ZS * zi:ZS * zi + ZS, YS * yi:YS * yi + YS] = \
                results[k]["out"]
            k += 1
    out = np.empty((1, S, S, S, C), np.float32)
    out[0, ..., 0] = np.clip(np.asarray(x, np.float32)[0, ..., 0], 0.0, MAXP)
    out[0, ..., 1:] = np.moveaxis(full, 0, -1)
    return out


def kernel(x, kernels, m, s, T, c0, c1):
    from concourse import bass_utils

    nc = _get_nc()
    in_maps = _host_prep(x, kernels, m, s, T, c0, c1)
    res = bass_utils.run_bass_kernel_spmd(nc, in_maps, list(range(NCORES)))
    return _gather(res.results, x)


# revision 47
# speedup vs baseline: 1.0870x; 1.0461x over previous
"""Trainium2 Bass kernel for nn_CppnPotentialCA (CPPN potential cellular automaton).

Reference computation (shapes hardcoded):
  x       [1,96,96,96,9] f32   potential field
  kernels [64,5,5,5]     f32   cross-channel conv kernels (normalized by sum)
  m, s    [64]           f32   Gaussian growth center / width
  T       []             f32   temperature
  c0, c1  [64]           i32   source / target channel per kernel pair

  kn   = kernels / sum(kernels)                  (per kernel, if sum > 0)
  pot  = conv3d_valid(wrap_pad(x)[c0[p]], kn[p]) for each pair p   [64,96,96,96]
  g    = exp(-(pot-m)^2 / (2 s^2)) * 2 - 1
  out  = clip(x + segment_sum(g, c1)/T, 0, 10)

Sharding: 2D data-parallel, 4 cores over z (24 planes each) x 2 cores over y
(48 rows each); toroidal halos of 2 resolved by host-side padding.

Device mapping (per core):
  - conv as dense matmul with z-offset blocking: output partitions =
    (16 pairs sharing 2 source channels, 8 z-offsets) = 128; contraction =
    (2 ci, 12 z-planes, 5 dy) = 120 SBUF partitions; dx in {0..4} via free-dim
    AP offsets -> only 5 accumulating matmuls per PSUM tile (vs 15 for the
    z-pair scheme).  free dim = 4 y-rows x 96 = 384.
  - growth via ScalarE: u = Square(pot*a + b), g' = Exp(-u + ln(2/T)) (fp16),
    Exp batched over the 4 channel-groups.
  - segment-sum as matmul with 0/1 matrix E accumulated over the 4 channel
    groups into PSUM [ (8 target ch, 8 zo), 384 ].
  - out = clip(xmod + seg, 0, 10) on VectorE, xmod = x - cnt_c/T from host.
Channel 0 never participates (c0,c1 in 1..8); its output x[...,0] is emitted
on the host.  All numeric inputs are runtime data; the compiled program is
value-independent.
"""

import numpy as np

C = 9        # channels (channel 0 idle)
S = 96       # spatial side
P = 64       # kernel pairs
K = 5        # kernel side
PAD = 2
MAXP = 10.0
NCORES = 8
NZC = 4                   # cores along z
NYC = 2                   # cores along y
ZS = S // NZC             # z planes per core = 24
YS = S // NYC             # y rows per core = 48
ZSLAB = ZS + 2 * PAD      # padded input z planes per core = 28
YSLAB = YS + 2 * PAD      # padded input y rows per core = 52
XW = S + 2 * PAD          # padded x width = 100
ZO = 8                    # z-offsets per output block
NZB = ZS // ZO            # 3 z-chunks per core
NCG = 4                   # channel groups (2 source channels each)
TCH = 12                  # input planes per chunk = ZO + K - 1
KROW = 2 * TCH * K        # data contraction rows = 120
KR = KROW + 1             # + ones-row carrying the folded Gaussian bias
YB = 4                    # y-rows per matmul tile
NSP = YS // YB            # 12 spatial tiles per chunk
FREE = YB * S             # matmul free dim = 384
ROWLEN = YS * XW          # im2col row length = 4800


def _build_nc():
    from contextlib import ExitStack

    import concourse.bass as bass
    import concourse.tile as tile
    from concourse import bacc, mybir
    from concourse.tile import add_dep_helper

    f32 = mybir.dt.float32
    f16 = mybir.dt.float16
    AF = mybir.ActivationFunctionType
    ALU = mybir.AluOpType

    nc = bacc.Bacc("TRN2", target_bir_lowering=False, debug=False,
                   num_devices=NCORES)

    # im2col input: xim[zb*4+cg, (ci2,t,dy), 4800] where row (ci2,t,dy) =
    # wrap-padded plane (8*zb + t) of group channel ci2, y rows dy..dy+47.
    xim_d = nc.dram_tensor("xim", [NZB * NCG, KR, ROWLEN], f16,
                           kind="ExternalInput")
    xmod_d = nc.dram_tensor("xmod", [C - 1, ZS, YS, S], f32,
                            kind="ExternalInput")
    w_d = nc.dram_tensor("wmat", [KR, NCG * K, 128], f16,
                         kind="ExternalInput")
    e_d = nc.dram_tensor("emat", [128, NCG * 128], f16, kind="ExternalInput")
    out_d = nc.dram_tensor("out", [C - 1, ZS, YS, S], f32,
                           kind="ExternalOutput")

    XM_C = ZS * YS * S       # xmod/out channel stride
    XM_Z = YS * S            # xmod/out z stride = 4608

    with tile.TileContext(nc) as tc, ExitStack() as ctx:
        consts = ctx.enter_context(tc.tile_pool(name="consts", bufs=1))
        rpool = ctx.enter_context(tc.tile_pool(name="rtiles", bufs=11))
        gpool = ctx.enter_context(tc.tile_pool(name="growth", bufs=3))
        xpool = ctx.enter_context(tc.tile_pool(name="xin", bufs=2))
        opool = ctx.enter_context(tc.tile_pool(name="oout", bufs=2))
        pconv = ctx.enter_context(tc.tile_pool(name="pconv", bufs=5,
                                               space="PSUM"))
        pseg = ctx.enter_context(tc.tile_pool(name="pseg", bufs=3,
                                              space="PSUM"))

        # Load DMAs alternate between two DGE rings (sync HWDGE / gpsimd
        # SWDGE) with ONE global window-4 completion chain, so at most 4
        # transfers share SDMA bandwidth and they finish in priority order.
        # The scalar ring is kept wait-free (its queue also runs the erfs).
        chain_hist = []

        def chain(ring, inst):
            ins = getattr(inst, "ins", inst)
            if len(chain_hist) >= 4:
                add_dep_helper(ins, chain_hist[-4], sync=True,
                               reason="window-serialize load DMAs")
            chain_hist.append(ins)

        WCOLS = NCG * K * 128        # 2560
        WCH = WCOLS // 4             # w chunk: 5 weight sets

        # PE warm-up: dummy matmuls on a zeroed tile keep the PE busy during
        # the initial DMA wait so the HAM clock-gate opens before real work.
        dumw = consts.tile([128, FREE], f16)
        nc.vector.memset(dumw[:, :], 0.0)
        dpc = pconv.tile([128, FREE], f32, name="dummy", tag="pc")
        for _ in range(12):
            nc.tensor.matmul(dpc[:, :], dumw[:, 0:128], dumw[:, :],
                             start=True, stop=True)

        w_sb = consts.tile([KR, WCOLS], f16)
        e_sb = consts.tile([128, NCG * 128], f16)

        def w_chunk(c):
            return nc.sync.dma_start(
                w_sb[:, c * WCH:(c + 1) * WCH],
                bass.AP(tensor=w_d, offset=c * WCH,
                        ap=[[WCOLS, KR], [1, WCH]]))

        def rt_part(eng, idx, rt, r0, r1):
            return eng.dma_start(
                rt[:, r0:r1],
                bass.AP(tensor=xim_d, offset=idx * KR * ROWLEN + r0,
                        ap=[[ROWLEN, KR], [1, r1 - r0]]))

        def xz_part(eng, zb, xz, q0, q1):
            return eng.dma_start(
                xz[:, q0:q1],
                bass.AP(tensor=xmod_d, offset=zb * ZO * XM_Z + q0,
                        ap=[[XM_C, C - 1], [XM_Z, ZO], [1, q1 - q0]]))

        # --- issue all loads up front, window-chained in priority order ---
        rtiles = []
        for zb in range(NZB):
            row = []
            for cg in range(NCG):
                rt = rpool.tile([KR, ROWLEN], f16, name=f"rt{zb}_{cg}",
                                tag="rt")
                row.append(rt)
            rtiles.append(row)
        xzs = [xpool.tile([P, YS * S], f32, name=f"xz{zb}", tag="xz")
               for zb in range(NZB)]

        Q = ROWLEN // 4
        xz_part(nc.scalar, 0, xzs[0], 0, YS * S)
        # ring A: weight chunks first (small), then even-q needs; ring C
        # starts with the very first compute dependency rtq0(cg0).
        chain("C", rt_part(nc.gpsimd, 0, rtiles[0][0], 0, Q))
        chain("A", w_chunk(0))
        chain("A", w_chunk(1))
        chain("C", rt_part(nc.gpsimd, 1, rtiles[0][1], 0, Q))
        chain("A", w_chunk(2))
        chain("A", w_chunk(3))
        chain("C", rt_part(nc.gpsimd, 2, rtiles[0][2], 0, Q))
        chain("A", rt_part(nc.sync, 3, rtiles[0][3], 0, Q))
        chain("C", nc.gpsimd.dma_start(e_sb[:, :], e_d.ap()))
        for q in range(1, 4):
            chain("C", rt_part(nc.gpsimd, 0, rtiles[0][0], q * Q,
                               (q + 1) * Q))
            chain("A", rt_part(nc.sync, 1, rtiles[0][1], q * Q, (q + 1) * Q))
            chain("C", rt_part(nc.gpsimd, 2, rtiles[0][2], q * Q,
                               (q + 1) * Q))
            chain("A", rt_part(nc.sync, 3, rtiles[0][3], q * Q, (q + 1) * Q))
        for zb in range(1, NZB):
            for cg in range(2):
                chain("A", rt_part(nc.sync, zb * NCG + cg, rtiles[zb][cg],
                                   0, ROWLEN))
            for cg in range(2, NCG):
                chain("C", rt_part(nc.gpsimd, zb * NCG + cg, rtiles[zb][cg],
                                   0, ROWLEN))
            chain("C", xz_part(nc.gpsimd, zb, xzs[zb], 0, YS * S))

        def conv_erf(rts, gt, sp, cg):
            pc = pconv.tile([128, FREE], f32, name=f"pc{cg}", tag="pc")
            for dx in range(K):
                si = cg * K + dx
                nc.tensor.matmul(
                    pc[:, :],
                    w_sb[:, si * 128:(si + 1) * 128],
                    rts[cg][0:KR, sp * YB:(sp + 1) * YB, dx:dx + S],
                    start=(dx == 0), stop=(dx == K - 1),
                )
            # growth: Derivative_Erf(v) = (2/sqrt(pi)) exp(-v^2);
            # the sqrt(pi)/T factor is folded into E.
            return nc.scalar.activation(
                gt[:, cg * FREE:(cg + 1) * FREE], pc[:, :], AF.Derivative_Erf)

        def seg_out(gt, last_erf, oz, xz, zoff, sp):
            ps = pseg.tile([128, FREE], f32, name="ps", tag="ps")
            for cg in range(NCG):
                seg = nc.tensor.matmul(
                    ps[:, :],
                    e_sb[:, cg * 128:(cg + 1) * 128],
                    gt[:, cg * FREE:(cg + 1) * FREE],
                    start=(cg == 0), stop=(cg == NCG - 1),
                )
                # group the 4 seg matmuls: all wait for the last erf so
                # the PE runs them back-to-back (one weight-mode switch)
                if cg < NCG - 1:
                    add_dep_helper(getattr(seg, "ins", seg),
                                   getattr(last_erf, "ins", last_erf),
                                   sync=True, reason="group seg matmuls")
            ysl = slice(sp * FREE, (sp + 1) * FREE)
            nc.vector.tensor_add(oz[:, ysl], ps[0:P, :], xz[:, ysl])
            nc.vector.tensor_scalar(
                oz[:, ysl], oz[:, ysl], 0.0, MAXP,
                op0=ALU.max, op1=ALU.min,
            )
            if sp % 3 == 2:
                # quarter-granular store so the tail transfer is short
                q0 = (sp - 2) * FREE
                nc.scalar.dma_start(
                    bass.AP(tensor=out_d, offset=zoff + q0,
                            ap=[[XM_C, C - 1], [XM_Z, ZO], [1, 3 * FREE]]),
                    oz[:, q0:q0 + 3 * FREE],
                )

        for zb in range(NZB):
            rts = [rt[:, :].rearrange("p (y x) -> p y x", y=YS, x=XW)
                   for rt in rtiles[zb]]
            zoff = zb * ZO * XM_Z
            xz = xzs[zb]
            oz = opool.tile([P, YS * S], f32, name=f"oz{zb}", tag="oz")
            if zb == 0:
                # cg-major within each quarter: compute starts as soon as
                # the first weight chunk + first tile quarter have landed
                for q in range(4):
                    sps = range(3 * q, 3 * q + 3)
                    gts = {sp: gpool.tile([128, NCG * FREE], f16,
                                          name=f"gt{sp}", tag="gt")
                           for sp in sps}
                    last = {}
                    for cg in range(NCG):
                        for sp in sps:
                            last[sp] = conv_erf(rts, gts[sp], sp, cg)
                    for sp in sps:
                        seg_out(gts[sp], last[sp], oz, xz, zoff, sp)
            else:
                for sp in range(NSP):
                    gt = gpool.tile([128, NCG * FREE], f16, name=f"gt{sp}",
                                    tag="gt")
                    last = None
                    for cg in range(NCG):
                        last = conv_erf(rts, gt, sp, cg)
                    seg_out(gt, last, oz, xz, zoff, sp)
    nc.compile()
    return nc


def _host_prep(x, kernels, m, s, T, c0, c1):
    x = np.asarray(x, np.float32)
    kernels = np.asarray(kernels, np.float32)
    m = np.asarray(m, np.float32)
    s = np.asarray(s, np.float32)
    Tf = np.float32(T)
    c0 = np.asarray(c0).astype(np.int64)
    c1 = np.asarray(c1).astype(np.int64)

    xt = np.ascontiguousarray(np.moveaxis(x[0], -1, 0))            # [9,96,96,96]
    ksum = kernels.sum(axis=(1, 2, 3), keepdims=True)
    kn = np.where(ksum > 0, kernels / ksum, kernels).astype(np.float32)

    xpad = np.pad(xt, ((0, 0), (PAD, PAD), (PAD, PAD), (PAD, PAD)),
                  mode="wrap").astype(np.float16)                  # [9,100,100,100]

    cnt = np.zeros(C, np.float32)
    for p in range(P):
        cnt[c1[p]] += 1.0
    xmod = (xt[1:] - (cnt[1:] / Tf)[:, None, None, None]).astype(np.float32)

    # channel groups: cg covers source channels {2cg+1, 2cg+2}; pairs within a
    # group ordered by p.  Output partition for pair index pp = pp*8 + zo.
    groups = [[p for p in range(P) if c0[p] in (2 * cg + 1, 2 * cg + 2)]
              for cg in range(NCG)]

    # Conv weights (Gaussian affine folded in): psum accumulates
    # v = a*pot + b with a = 1/(s*sqrt2), b = -m*a.  The ones-row (row KROW)
    # carries b split into f16(b) + residual across dx slots 0 and 1.
    W = np.zeros((KR, NCG * K, 128), np.float32)
    E = np.zeros((128, NCG * 128), np.float16)
    a = (1.0 / (s * np.sqrt(np.float32(2.0)))).astype(np.float32)
    b = (-m * a).astype(np.float32)
    for cg in range(NCG):
        for pp, p in enumerate(groups[cg]):
            ci2 = 0 if c0[p] == 2 * cg + 1 else 1
            for zo in range(ZO):
                part = pp * ZO + zo
                for t in range(TCH):
                    d = t - zo
                    if 0 <= d < K:
                        W[ci2 * TCH * K + t * K:ci2 * TCH * K + t * K + K,
                          cg * K:cg * K + K, part] = kn[p, d] * a[p]
                bh = np.float16(b[p])
                W[KROW, cg * K + 0, part] = bh
                W[KROW, cg * K + 1, part] = b[p] - np.float32(bh)
                # seg-sum target: (c1-1)*8 + zo; sqrt(pi)/T rescales
                # Derivative_Erf(v) = (2/sqrt(pi)) exp(-v^2) to (2/T) exp(-v^2)
                E[part, cg * 128 + (int(c1[p]) - 1) * ZO + zo] = np.float16(
                    np.sqrt(np.pi) / Tf)
    W = W.astype(np.float16)

    in_maps = []
    for zi in range(NZC):
        for yi in range(NYC):
            slab = xpad[:, ZS * zi:ZS * zi + ZSLAB,
                        YS * yi:YS * yi + YSLAB]                   # [9,28,52,100]
            xim = np.empty((NZB * NCG, KR, ROWLEN), np.float16)
            xim[:, KROW] = np.float16(1.0)
            for zb in range(NZB):
                for cg in range(NCG):
                    dst = xim[zb * NCG + cg, :KROW].reshape(2, TCH, K, ROWLEN)
                    for ci2 in range(2):
                        ch = 2 * cg + 1 + ci2
                        for dy in range(K):
                            dst[ci2, :, dy] = (
                                slab[ch, zb * ZO:zb * ZO + TCH,
                                     dy:dy + YS].reshape(TCH, ROWLEN))
            in_maps.append({
                "xim": xim,
                "xmod": np.ascontiguousarray(
                    xmod[:, ZS * zi:ZS * zi + ZS, YS * yi:YS * yi + YS]),
                "wmat": W,
                "emat": E,
            })
    return in_maps


_NC_CACHE = {}


def _get_nc():
    if "nc" not in _NC_CACHE:
        _NC_CACHE["nc"] = _build_nc()
    return _NC_CACHE["nc"]


def _gather(results, x):
    full = np.empty((C - 1, S, S, S), np.float32)
    k = 0
    for zi in range(NZC):
        for yi in range(NYC):
            full[:, ZS * zi:ZS * zi + ZS, YS * yi:YS * yi + YS] = \
                results[k]["out"]
            k += 1
    out = np.empty((1, S, S, S, C), np.float32)
    out[0, ..., 0] = np.clip(np.asarray(x, np.float32)[0, ..., 0], 0.0, MAXP)
    out[0, ..., 1:] = np.moveaxis(full, 0, -1)
    return out


def kernel(x, kernels, m, s, T, c0, c1):
    from concourse import bass_utils

    nc = _get_nc()
    in_maps = _host_prep(x, kernels, m, s, T, c0, c1)
    res = bass_utils.run_bass_kernel_spmd(nc, in_maps, list(range(NCORES)))
    return _gather(res.results, x)
